# revision 30
# baseline (speedup 1.0000x reference)
"""Trainium2 Bass kernel for CausalAnalysisPredictor (gnn_message_passing).

kernel(**inputs) takes the FULL unsharded inputs and returns the FULL
[16384, 51] float32 output. Relations are sorted by head object on the host
and sharded contiguously across 8 NeuronCores. The head half of the folded
post_cat contraction exploits the object-level structure: per-object rows
A = edge_ctx @ Wfold_head are computed once per core (~640 objects) and
expanded to relations with block-one-hot E matmuls (fixed column windows,
host-zero-padded so the same instruction stream is valid on every core).
The tail half stays a per-relation dense matmul on host-gathered context.
"""

import os
import sys
import types

import numpy as np

try:
    import concourse  # noqa: F401
except ImportError:  # pragma: no cover
    sys.path.insert(0, "/opt/trn_rl_repo")

import ml_dtypes

import concourse.mybir as mybir
import concourse.tile as tile
from concourse import bacc
from concourse.bass_utils import run_bass_kernel_spmd

BF16 = mybir.dt.bfloat16
F32 = mybir.dt.float32
NPBF16 = ml_dtypes.bfloat16

N_OBJ, N_REL = 4096, 16384
H, P = 512, 4096
NOC, NRC = 151, 51
NCORES = 8
NRELC = N_REL // NCORES  # 2048 relations per core
KC = H // 128            # 4 feat chunks (spt1 hidden & per-side edge ctx)
MO = P // 128            # 32 output-feature chunks
NCH = NRELC // 512       # 4 relation chunks of 512
GOFF = 64                # partition offset of the gate/vis/freq lane block
OBC = 5                  # head-object 128-chunks per core (span <= 640)


def _windows(sl, sr):
    """Expansion windows [512g - sl, 512(g+1) + sr) in relation-column space.

    sl/sr bound how far any group's true column range can start before /
    end after its nominal 512-aligned slot (measured from the input on the
    host; the same values must hold on every core for the shared program).
    """
    wins = [
        (max(0, 512 * g - sl), min(NRELC, 512 * (g + 1) + sr)) for g in range(OBC)
    ]
    wofs = [0]
    for lo, hi in wins:
        wofs.append(wofs[-1] + (hi - lo))
    parts = []
    for n in range(NCH):
        c0, c1 = 512 * n, 512 * n + 512
        ps = []
        for g in range(OBC):
            lo, hi = wins[g]
            a, b = max(lo, c0), min(hi, c1)
            if a < b:
                ps.append((g, a, b))
        ps.sort(key=lambda p: -(p[2] - p[1]))  # full 512-part first (start=True)
        assert ps[0][2] - ps[0][1] == 512
        parts.append(ps)
    return wins, wofs, parts

AF = mybir.ActivationFunctionType
ALU = mybir.AluOpType

last_exec_time_ns = None  # set when BASS_KERNEL_TRACE=1


def _register_ntff_hook():
    if "antenv.axon_hooks" in sys.modules:
        return
    hook = None
    try:
        from trn_agent_boot.trn_boot import _ntff_profile_via_ctypes

        hook = _ntff_profile_via_ctypes("/opt/axon/libaxon_pjrt.so")
    except Exception:
        hook = None
    mod = types.ModuleType("antenv.axon_hooks")
    mod.get_axon_ntff_profile_hook = lambda: hook
    mod.set_axon_ntff_profile_hook = lambda h: None
    sys.modules["antenv.axon_hooks"] = mod


_nc_cache = {}


def _build(sl, sr):
    if (sl, sr) in _nc_cache:
        return _nc_cache[(sl, sr)]
    wins, wofs, parts = _windows(sl, sr)
    ecols = wofs[-1]

    nc = bacc.Bacc("TRN2", target_bir_lowering=False, debug=False, num_devices=NCORES)

    # ---- DRAM parameters (per-core shards / replicated tables) ----
    eTd = nc.declare_dram_parameter("eTd", [KC, 128, NRELC], BF16, isOutput=False)
    # one [128, KC*OBC*128] tile (k-major columns) -> 128 large descriptors
    ectxTo = nc.declare_dram_parameter("ectxTo", [128, KC * OBC * 128], BF16, isOutput=False)
    wfh = nc.declare_dram_parameter("wfh", [KC, 128, P], BF16, isOutput=False)
    Ed = nc.declare_dram_parameter("Ed", [128, ecols], BF16, isOutput=False)
    gfTd = nc.declare_dram_parameter("gfTd", [NRC, NRELC], F32, isOutput=False)
    # row 32 of bboxT is all-ones and row 32 of wspt1 is b_spt1 (bias fold)
    bboxT = nc.declare_dram_parameter("bboxT", [33, NRELC], BF16, isOutput=False)
    uT = nc.declare_dram_parameter("uT", [P, NRELC], BF16, isOutput=False)
    # per-m merged stream: [tail wcat | wspt2] -> one DMA per (n, m)
    wcs = nc.declare_dram_parameter("wcs", [MO, 128, 2 * KC * 128], BF16, isOutput=False)
    wspt1 = nc.declare_dram_parameter("wspt1", [33, H], BF16, isOutput=False)
    wcg = nc.declare_dram_parameter("wcg", [128, MO * 128], BF16, isOutput=False)
    wvisp = nc.declare_dram_parameter("wvisp", [128, MO * 128], BF16, isOutput=False)
    # packed biases: cols [0:32]=bcat, [32:64]=bs2, [64]=bctx
    bpack = nc.declare_dram_parameter("bpack", [128, 2 * MO + 1], F32, isOutput=False)
    out_t = nc.declare_dram_parameter("out_t", [NRC, NRELC], F32, isOutput=True)

    with tile.TileContext(nc) as tc:
        with (
            tc.tile_pool(name="sbuf", bufs=1) as pool,
            tc.tile_pool(name="psum", bufs=1, space="PSUM") as pp,
        ):
            # ---- phase-0 loads: spt1 + A-phase inputs lead their queues ----
            wspt1_t = pool.tile([33, H], BF16)
            nc.scalar.dma_start(wspt1_t[:], wspt1[:])
            bboxT_t = pool.tile([33, NRELC], BF16)
            nc.scalar.dma_start(bboxT_t[:], bboxT[:])
            ectxTo_t = pool.tile([128, KC * OBC * 128], BF16)
            nc.sync.dma_start(ectxTo_t[:], ectxTo[:])
            bp_t = pool.tile([128, 2 * MO + 1], F32)
            nc.scalar.dma_start(bp_t[:], bpack[:])
            # wfh: first column block small on sync (unblocks the A phase);
            # the bulk rides the scalar queue in large-descriptor DMAs
            wfh_t = [pool.tile([128, P], BF16, name=f"wfh{k}") for k in range(KC)]
            for k in range(KC):
                nc.sync.dma_start(wfh_t[k][:, 0:512], wfh[k][:, 0:512])
            for k in range(KC):
                nc.scalar.dma_start(wfh_t[k][:, 512:P], wfh[k][:, 512:P])
            E_t = pool.tile([128, ecols], BF16)
            nc.gpsimd.dma_start(E_t[:], Ed[:])
            eT = [pool.tile([128, NRELC], BF16, name=f"eT{k}") for k in range(KC)]
            for k in range(KC):
                eng = nc.scalar if k < 2 else nc.gpsimd
                eng.dma_start(eT[k][:], eTd[k])
            wcg_t = pool.tile([128, MO, 128], BF16)
            nc.gpsimd.dma_start(wcg_t[:], wcg[:].rearrange("p (m c) -> p m c", m=MO))
            wvis_t = pool.tile([128, MO, 128], BF16)
            nc.gpsimd.dma_start(wvis_t[:], wvisp[:].rearrange("p (m c) -> p m c", m=MO))
            gfT_t = pool.tile([128, NRELC], F32)
            nc.gpsimd.dma_start(gfT_t[GOFF : GOFF + NRC, :], gfTd[:])

            # ---- spt1 (bbox only; warms the PE while DMAs stream) ----
            s1T = [pool.tile([128, NRELC], BF16, name=f"s1T{k}") for k in range(KC)]
            for k in range(KC):
                for n in range(NCH):
                    ps = pp.tile([128, 512], F32, tag="cat", bufs=3)
                    nc.tensor.matmul(
                        ps[:],
                        wspt1_t[:, k * 128 : (k + 1) * 128],
                        bboxT_t[:, n * 512 : (n + 1) * 512],
                        start=True,
                        stop=True,
                    )
                    nc.scalar.activation(
                        s1T[k][:, n * 512 : (n + 1) * 512], ps[:], AF.Relu
                    )

            # ---- chunk-0 spt2 warm-up: fills the DMA staging window with PE
            # work that depends only on on-chip s1T + small weight loads ----
            PRESPT = 6
            wcs_pre = {}
            for m in range(PRESPT):
                t = pool.tile([128, 2 * KC * 128], BF16, tag="wcs_b", bufs=10)
                nc.sync.dma_start(t[:], wcs[m])
                wcs_pre[m] = t
            r2_store = {}
            for m in range(PRESPT):
                ps_spt = pp.tile([128, 512], F32, tag="spt", bufs=2)
                for k in range(KC):
                    nc.tensor.matmul(
                        ps_spt[:],
                        wcs_pre[m][:, (KC + k) * 128 : (KC + k + 1) * 128],
                        s1T[k][:, 0:512],
                        start=(k == 0),
                        stop=(k == KC - 1),
                    )
                r2 = pool.tile([128, 512], BF16, tag="r2", bufs=9)
                nc.vector.tensor_scalar(
                    out=r2[:],
                    in0=ps_spt[:],
                    scalar1=bp_t[:, MO + m : MO + m + 1],
                    scalar2=0.0,
                    op0=ALU.add,
                    op1=ALU.max,
                )
                r2_store[m] = r2

            # ---- A phase: per-object head reps A[g] = ectx_chunk @ Wfold_h ----
            # fs-outer so each freshly-arrived wfh column block feeds OBC
            # matmul groups before the next block is needed (DMA pipelining)
            A = [pool.tile([128, P], BF16, name=f"A{g}") for g in range(OBC)]
            for fs in range(8):
                fsl = slice(fs * 512, (fs + 1) * 512)
                for g in range(OBC):
                    ps = pp.tile([128, 512], F32, tag="cat", bufs=3)
                    for k in range(KC):
                        osl = slice(k * OBC * 128 + g * 128, k * OBC * 128 + (g + 1) * 128)
                        nc.tensor.matmul(
                            ps[:],
                            ectxTo_t[:, osl],
                            wfh_t[k][:, fsl],
                            start=(k == 0),
                            stop=(k == KC - 1),
                        )
                    nc.scalar.activation(A[g][:, fsl], ps[:], AF.Copy)

            outT = pool.tile([128, NRELC], F32)

            for n in range(NCH):
                nsl = slice(n * 512, (n + 1) * 512)
                psum_cg = pp.tile([128, 512], F32, tag="cg", bufs=2)
                lag = []  # (pc, u_b, m) awaiting their cg/vis matmuls
                for m in range(MO + 1):
                    if m < MO:
                        if n == 0 and m < PRESPT:
                            wcs_b = wcs_pre[m]
                        else:
                            wcs_b = pool.tile(
                                [128, 2 * KC * 128], BF16, tag="wcs_b", bufs=10
                            )
                            nc.sync.dma_start(wcs_b[:], wcs[m])
                        wcat_b = wcs_b[:, 0 : KC * 128]
                        wspt2_b = wcs_b[:, KC * 128 : 2 * KC * 128]
                        u_b = pool.tile([128, 512], BF16, tag="u_b", bufs=6)
                        nc.scalar.dma_start(u_b[:], uT[m * 128 : (m + 1) * 128, nsl])
                        msl = slice(m * 128, (m + 1) * 128)
                        ps_cat = pp.tile([128, 512], F32, tag="cat", bufs=3)
                        # head contribution: expansion matmuls over A (full
                        # 512-part first: its start=True zeroes the chunk)
                        for i, (g, a, b) in enumerate(parts[n]):
                            lo = wins[g][0]
                            nc.tensor.matmul(
                                ps_cat[:, a - 512 * n : b - 512 * n],
                                A[g][:, msl],
                                E_t[:, wofs[g] + (a - lo) : wofs[g] + (b - lo)],
                                start=(i == 0),
                                stop=False,
                                skip_group_check=True,
                            )
                        # tail contribution: dense per-relation matmul
                        for k in range(KC):
                            nc.tensor.matmul(
                                ps_cat[:],
                                wcat_b[:, k * 128 : (k + 1) * 128],
                                eT[k][:, nsl],
                                start=False,
                                stop=(k == KC - 1),
                                skip_group_check=True,
                            )
                        if n == 0 and m < PRESPT:
                            r2 = r2_store.pop(m)
                        else:
                            ps_spt = pp.tile([128, 512], F32, tag="spt", bufs=2)
                            for k in range(KC):
                                nc.tensor.matmul(
                                    ps_spt[:],
                                    wspt2_b[:, k * 128 : (k + 1) * 128],
                                    s1T[k][:, nsl],
                                    start=(k == 0),
                                    stop=(k == KC - 1),
                                )
                            r2 = pool.tile([128, 512], BF16, tag="r2", bufs=9)
                            nc.vector.tensor_scalar(
                                out=r2[:],
                                in0=ps_spt[:],
                                scalar1=bp_t[:, MO + m : MO + m + 1],
                                scalar2=0.0,
                                op0=ALU.add,
                                op1=ALU.max,
                            )
                        r1 = pool.tile([128, 512], BF16, tag="r1", bufs=3)
                        nc.scalar.activation(
                            r1[:], ps_cat[:], AF.Relu, bias=bp_t[:, m : m + 1]
                        )
                        pc = pool.tile([128, 512], BF16, tag="pc", bufs=4)
                        nc.vector.tensor_mul(out=pc[:], in0=r1[:], in1=r2[:])
                        lag.append((pc, u_b, m))
                    while lag and (len(lag) > 2 or m == MO):
                        pc_l, u_l, m_l = lag.pop(0)
                        nc.tensor.matmul(
                            psum_cg[:],
                            wcg_t[:, m_l, :],
                            pc_l[:],
                            start=(m_l == 0),
                            stop=False,
                            skip_group_check=True,
                        )
                        nc.tensor.matmul(
                            psum_cg[:],
                            wvis_t[:, m_l, :],
                            u_l[:],
                            start=False,
                            stop=(m_l == MO - 1),
                            skip_group_check=True,
                        )

                # -- epilogue: rel^T = (ctx + b_ctx) * sigmoid(vis+gate+frq) --
                sarg = pool.tile([128, 512], F32, tag="sarg", bufs=2)
                nc.vector.tensor_add(
                    out=sarg[GOFF : GOFF + NRC, :],
                    in0=psum_cg[GOFF : GOFF + NRC, :],
                    in1=gfT_t[GOFF : GOFF + NRC, nsl],
                )
                sg = pool.tile([128, 512], BF16, tag="sg", bufs=2)
                nc.scalar.activation(
                    sg[0:NRC, :], sarg[GOFF : GOFF + NRC, :], AF.Sigmoid
                )
                nc.vector.scalar_tensor_tensor(
                    out=outT[0:NRC, nsl],
                    in0=psum_cg[0:NRC, :],
                    scalar=bp_t[0:NRC, 2 * MO : 2 * MO + 1],
                    in1=sg[0:NRC, :],
                    op0=ALU.add,
                    op1=ALU.mult,
                )
                nc.sync.dma_start(out_t[:, nsl], outT[0:NRC, nsl])

    nc.compile()
    _nc_cache[(sl, sr)] = nc
    return nc


def _prep_core(inputs, c, common):
    perm = common["_perm"]
    sl = perm[c * NRELC : (c + 1) * NRELC]
    pi = np.asarray(inputs["pair_idx"])[sl].astype(np.int64)
    pp_ = np.asarray(inputs["pair_pred"])[sl].astype(np.int64)
    bbox = np.asarray(inputs["pair_bbox"])[sl].astype(np.float32)
    uf = np.asarray(inputs["union_features"])[sl].astype(np.float32)

    ectx = common["_ectx_bf16"]
    h = pi[:, 0]
    base = (int(h[0]) // 128) * 128
    assert int(h[-1]) < base + OBC * 128, "head span exceeds OBC chunks"

    # object slab, transposed, k-major single tile: [128, KC*OBC*128]
    eo = np.zeros((OBC * 128, H), dtype=NPBF16)
    hi_obj = min(base + OBC * 128, N_OBJ)
    eo[: hi_obj - base] = ectx[base:hi_obj]
    ectxTo = np.ascontiguousarray(
        eo.T.reshape(KC, 128, OBC * 128).transpose(1, 0, 2).reshape(128, KC * OBC * 128)
    )

    # block one-hot expansion matrix with fixed windows
    wins, wofs = common["_wins"], common["_wofs"]
    E = np.zeros((128, wofs[-1]), dtype=NPBF16)
    g_all = (h - base) // 128
    for j in range(NRELC):
        g = int(g_all[j])
        lo, hi = wins[g]
        assert lo <= j < hi, "relation outside its group's fixed window"
        E[int(h[j] - base) % 128, wofs[g] + (j - lo)] = 1.0

    e_tail = ectx[pi[:, 1]]  # [NRELC, 512]
    eTd = np.ascontiguousarray(e_tail.T).reshape(KC, 128, NRELC)

    gf = common["_freq_f32"][pp_[:, 0] * NOC + pp_[:, 1]] + common["_bvg"]
    bboxT_l = np.ones((33, NRELC), dtype=np.float32)
    bboxT_l[:32] = bbox.T
    m = {
        "eTd": eTd,
        "ectxTo": ectxTo,
        "Ed": E,
        "gfTd": np.ascontiguousarray(gf.T.astype(np.float32)),
        "bboxT": np.ascontiguousarray(bboxT_l).astype(NPBF16),
        "uT": np.ascontiguousarray(uf.T).astype(NPBF16),
    }
    m.update({k: v for k, v in common.items() if not k.startswith("_")})
    return m


def _prep_common(inputs):
    f32 = lambda k: np.asarray(inputs[k], dtype=np.float32)

    perm = np.argsort(np.asarray(inputs["pair_idx"])[:, 0], kind="stable")
    # measure per-core group-boundary deviations to size expansion windows
    heads_s = np.asarray(inputs["pair_idx"])[perm, 0]
    devs = []
    for c in range(NCORES):
        h = heads_s[c * NRELC : (c + 1) * NRELC]
        base = (int(h[0]) // 128) * 128
        g = (h - base) // 128
        for gg in range(1, OBC):
            devs.append(int(np.searchsorted(g, gg)) - 512 * gg)
    sl = max(16, ((-min(devs) + 16 + 15) // 16) * 16)
    sr = max(16, ((max(devs) + 16 + 15) // 16) * 16)

    wemb = f32("W_post_emb")  # [512, 1024]
    wcat0 = f32("W_post_cat")  # [1024, 4096]
    # fold: ctx_rep @ W_post_cat == [Eh|Et] @ [[Wh@Wcat_top];[Wt@Wcat_bot]]
    wfold_h = wemb[:, :H] @ wcat0[:H]  # [512, 4096]
    wfold_t = wemb[:, H:] @ wcat0[H:]  # [512, 4096]
    wcat_l = wfold_t.reshape(KC, 128, MO, 128).transpose(2, 1, 0, 3).reshape(
        MO, 128, KC * 128
    )
    wfh_l = np.ascontiguousarray(wfold_h.reshape(KC, 128, P)).astype(NPBF16)

    wspt2 = f32("W_spt2")  # [512, 4096]
    wspt2_l = wspt2.reshape(KC, 128, MO, 128).transpose(2, 1, 0, 3).reshape(
        MO, 128, KC * 128
    )
    wcs_l = np.ascontiguousarray(
        np.concatenate([wcat_l, wspt2_l], axis=2)
    ).astype(NPBF16)

    wcg = np.zeros((P, 128), dtype=np.float32)
    wcg[:, :NRC] = f32("W_ctx")
    wcg[:, GOFF : GOFF + NRC] = f32("W_gate")
    wcg_l = np.ascontiguousarray(
        wcg.reshape(MO, 128, 128).transpose(1, 0, 2).reshape(128, MO * 128)
    ).astype(NPBF16)

    wvis = np.zeros((P, 128), dtype=np.float32)
    wvis[:, GOFF : GOFF + NRC] = f32("W_vis")
    wvis_l = np.ascontiguousarray(
        wvis.reshape(MO, 128, 128).transpose(1, 0, 2).reshape(128, MO * 128)
    ).astype(NPBF16)

    col = lambda b, n: np.ascontiguousarray(
        np.asarray(b, dtype=np.float32).reshape(n, 128).T
    )
    bpack_l = np.zeros((128, 2 * MO + 1), dtype=np.float32)
    bpack_l[:, 0:MO] = col(
        f32("b_post_emb")[:H] @ wcat0[:H]
        + f32("b_post_emb")[H:] @ wcat0[H:]
        + f32("b_post_cat"),
        MO,
    )
    bpack_l[:, MO : 2 * MO] = col(inputs["b_spt2"], MO)
    bpack_l[:NRC, 2 * MO] = f32("b_ctx")

    wspt1_l = np.zeros((33, H), dtype=np.float32)
    wspt1_l[:32] = f32("W_spt1")
    wspt1_l[32] = f32("b_spt1")

    wins, wofs, _ = _windows(sl, sr)
    return {
        "_perm": perm,
        "_sl": sl,
        "_sr": sr,
        "_wins": wins,
        "_wofs": wofs,
        "_ectx_bf16": f32("edge_ctx").astype(NPBF16),
        "_freq_f32": f32("freq_table"),
        "_bvg": (f32("b_vis") + f32("b_gate"))[None, :],
        "wcs": wcs_l,
        "wfh": wfh_l,
        "wspt1": wspt1_l.astype(NPBF16),
        "wcg": wcg_l,
        "wvisp": wvis_l,
        "bpack": bpack_l,
    }


def kernel(**inputs) -> np.ndarray:
    global last_exec_time_ns
    trace = bool(os.environ.get("BASS_KERNEL_TRACE"))
    if trace:
        _register_ntff_hook()
    common = _prep_common(inputs)
    nc = _build(common["_sl"], common["_sr"])
    in_maps = [_prep_core(inputs, c, common) for c in range(NCORES)]
    res = run_bass_kernel_spmd(nc, in_maps, list(range(NCORES)), trace=trace)
    if trace:
        last_exec_time_ns = res.exec_time_ns
    out_sorted = np.concatenate(
        [np.asarray(res.results[c]["out_t"]).T for c in range(NCORES)], axis=0
    ).astype(np.float32)
    out = np.empty_like(out_sorted)
    out[common["_perm"]] = out_sorted
    return np.ascontiguousarray(out)


# revision 31
# speedup vs baseline: 1.0242x; 1.0242x over previous
"""Trainium2 Bass kernel for CausalAnalysisPredictor (gnn_message_passing).

kernel(**inputs) takes the FULL unsharded inputs and returns the FULL
[16384, 51] float32 output. Relations are sorted by head object on the host
and sharded contiguously across 8 NeuronCores. The head half of the folded
post_cat contraction exploits the object-level structure: per-object rows
A = edge_ctx @ Wfold_head are computed once per core (~640 objects) and
expanded to relations with block-one-hot E matmuls (fixed column windows,
host-zero-padded so the same instruction stream is valid on every core).
The tail half stays a per-relation dense matmul on host-gathered context.
"""

import os
import sys
import types

import numpy as np

try:
    import concourse  # noqa: F401
except ImportError:  # pragma: no cover
    sys.path.insert(0, "/opt/trn_rl_repo")

import ml_dtypes

import concourse.mybir as mybir
import concourse.tile as tile
from concourse import bacc
from concourse.bass_utils import run_bass_kernel_spmd

BF16 = mybir.dt.bfloat16
F32 = mybir.dt.float32
NPBF16 = ml_dtypes.bfloat16

N_OBJ, N_REL = 4096, 16384
H, P = 512, 4096
NOC, NRC = 151, 51
NCORES = 8
NRELC = N_REL // NCORES  # 2048 relations per core
KC = H // 128            # 4 feat chunks (spt1 hidden & per-side edge ctx)
MO = P // 128            # 32 output-feature chunks
NCH = NRELC // 512       # 4 relation chunks of 512
GOFF = 64                # partition offset of the gate/vis/freq lane block
OBC = 5                  # head-object 128-chunks per core (span <= 640)


def _windows(sl, sr):
    """Expansion windows [512g - sl, 512(g+1) + sr) in relation-column space.

    sl/sr bound how far any group's true column range can start before /
    end after its nominal 512-aligned slot (measured from the input on the
    host; the same values must hold on every core for the shared program).
    """
    wins = [
        (max(0, 512 * g - sl), min(NRELC, 512 * (g + 1) + sr)) for g in range(OBC)
    ]
    wofs = [0]
    for lo, hi in wins:
        wofs.append(wofs[-1] + (hi - lo))
    parts = []
    for n in range(NCH):
        c0, c1 = 512 * n, 512 * n + 512
        ps = []
        for g in range(OBC):
            lo, hi = wins[g]
            a, b = max(lo, c0), min(hi, c1)
            if a < b:
                ps.append((g, a, b))
        ps.sort(key=lambda p: -(p[2] - p[1]))  # full 512-part first (start=True)
        assert ps[0][2] - ps[0][1] == 512
        parts.append(ps)
    return wins, wofs, parts

AF = mybir.ActivationFunctionType
ALU = mybir.AluOpType

last_exec_time_ns = None  # set when BASS_KERNEL_TRACE=1


def _register_ntff_hook():
    if "antenv.axon_hooks" in sys.modules:
        return
    hook = None
    try:
        from trn_agent_boot.trn_boot import _ntff_profile_via_ctypes

        hook = _ntff_profile_via_ctypes("/opt/axon/libaxon_pjrt.so")
    except Exception:
        hook = None
    mod = types.ModuleType("antenv.axon_hooks")
    mod.get_axon_ntff_profile_hook = lambda: hook
    mod.set_axon_ntff_profile_hook = lambda h: None
    sys.modules["antenv.axon_hooks"] = mod


_nc_cache = {}


def _build(sl, sr):
    if (sl, sr) in _nc_cache:
        return _nc_cache[(sl, sr)]
    wins, wofs, parts = _windows(sl, sr)
    ecols = wofs[-1]

    nc = bacc.Bacc("TRN2", target_bir_lowering=False, debug=False, num_devices=NCORES)

    # ---- DRAM parameters (per-core shards / replicated tables) ----
    eTd = nc.declare_dram_parameter("eTd", [KC, 128, NRELC], BF16, isOutput=False)
    # one [128, KC*OBC*128] tile (k-major columns) -> 128 large descriptors
    ectxTo = nc.declare_dram_parameter("ectxTo", [128, KC * OBC * 128], BF16, isOutput=False)
    wfh = nc.declare_dram_parameter("wfh", [KC, 128, P], BF16, isOutput=False)
    Ed = nc.declare_dram_parameter("Ed", [128, ecols], BF16, isOutput=False)
    gfTd = nc.declare_dram_parameter("gfTd", [NRC, NRELC], F32, isOutput=False)
    # row 32 of bboxT is all-ones and row 32 of wspt1 is b_spt1 (bias fold)
    bboxT = nc.declare_dram_parameter("bboxT", [33, NRELC], BF16, isOutput=False)
    uT = nc.declare_dram_parameter("uT", [P, NRELC], BF16, isOutput=False)
    # per-m merged stream: [tail wcat | wspt2] -> one DMA per (n, m)
    wcs = nc.declare_dram_parameter("wcs", [MO, 128, 2 * KC * 128], BF16, isOutput=False)
    wspt1 = nc.declare_dram_parameter("wspt1", [33, H], BF16, isOutput=False)
    wcg = nc.declare_dram_parameter("wcg", [128, MO * 128], BF16, isOutput=False)
    wvisp = nc.declare_dram_parameter("wvisp", [128, MO * 128], BF16, isOutput=False)
    # packed biases: cols [0:32]=bcat, [32:64]=bs2, [64]=bctx
    bpack = nc.declare_dram_parameter("bpack", [128, 2 * MO + 1], F32, isOutput=False)
    out_t = nc.declare_dram_parameter("out_t", [NRC, NRELC], F32, isOutput=True)

    with tile.TileContext(nc) as tc:
        with (
            tc.tile_pool(name="sbuf", bufs=1) as pool,
            tc.tile_pool(name="psum", bufs=1, space="PSUM") as pp,
        ):
            # ---- phase-0 loads: spt1 + A-phase inputs lead their queues ----
            wspt1_t = pool.tile([33, H], BF16)
            nc.scalar.dma_start(wspt1_t[:], wspt1[:])
            bboxT_t = pool.tile([33, NRELC], BF16)
            nc.scalar.dma_start(bboxT_t[:], bboxT[:])
            ectxTo_t = pool.tile([128, KC * OBC * 128], BF16)
            nc.sync.dma_start(ectxTo_t[:], ectxTo[:])
            bp_t = pool.tile([128, 2 * MO + 1], F32)
            nc.scalar.dma_start(bp_t[:], bpack[:])
            # wfh: first column block small on sync (unblocks the A phase);
            # the bulk rides the scalar queue in large-descriptor DMAs
            wfh_t = [pool.tile([128, P], BF16, name=f"wfh{k}") for k in range(KC)]
            for k in range(KC):
                nc.sync.dma_start(wfh_t[k][:, 0:512], wfh[k][:, 0:512])
            for k in range(KC):
                nc.scalar.dma_start(wfh_t[k][:, 512:P], wfh[k][:, 512:P])
            E_t = pool.tile([128, ecols], BF16)
            nc.gpsimd.dma_start(E_t[:], Ed[:])
            eT = [pool.tile([128, NRELC], BF16, name=f"eT{k}") for k in range(KC)]
            for k in range(KC):
                eng = nc.scalar if k < 2 else nc.gpsimd
                eng.dma_start(eT[k][:], eTd[k])
            wcg_t = pool.tile([128, MO, 128], BF16)
            nc.gpsimd.dma_start(wcg_t[:], wcg[:].rearrange("p (m c) -> p m c", m=MO))
            wvis_t = pool.tile([128, MO, 128], BF16)
            nc.gpsimd.dma_start(wvis_t[:], wvisp[:].rearrange("p (m c) -> p m c", m=MO))
            gfT_t = pool.tile([128, NRELC], F32)
            nc.gpsimd.dma_start(gfT_t[GOFF : GOFF + NRC, :], gfTd[:])

            # ---- spt1 (bbox only; warms the PE while DMAs stream) ----
            s1T = [pool.tile([128, NRELC], BF16, name=f"s1T{k}") for k in range(KC)]
            for k in range(KC):
                for n in range(NCH):
                    ps = pp.tile([128, 512], F32, tag="cat", bufs=3)
                    nc.tensor.matmul(
                        ps[:],
                        wspt1_t[:, k * 128 : (k + 1) * 128],
                        bboxT_t[:, n * 512 : (n + 1) * 512],
                        start=True,
                        stop=True,
                    )
                    nc.scalar.activation(
                        s1T[k][:, n * 512 : (n + 1) * 512], ps[:], AF.Relu
                    )

            # ---- A phase: per-object head reps A[g] = ectx_chunk @ Wfold_h ----
            # fs-outer so each freshly-arrived wfh column block feeds OBC
            # matmul groups before the next block is needed (DMA pipelining)
            A = [pool.tile([128, P], BF16, name=f"A{g}") for g in range(OBC)]
            for fs in range(8):
                fsl = slice(fs * 512, (fs + 1) * 512)
                for g in range(OBC):
                    ps = pp.tile([128, 512], F32, tag="cat", bufs=3)
                    for k in range(KC):
                        osl = slice(k * OBC * 128 + g * 128, k * OBC * 128 + (g + 1) * 128)
                        nc.tensor.matmul(
                            ps[:],
                            ectxTo_t[:, osl],
                            wfh_t[k][:, fsl],
                            start=(k == 0),
                            stop=(k == KC - 1),
                        )
                    nc.scalar.activation(A[g][:, fsl], ps[:], AF.Copy)

            outT = pool.tile([128, NRELC], F32)

            for n in range(NCH):
                nsl = slice(n * 512, (n + 1) * 512)
                psum_cg = pp.tile([128, 512], F32, tag="cg", bufs=2)
                lag = []  # (pc, u_b, m) awaiting their cg/vis matmuls
                for m in range(MO + 1):
                    if m < MO:
                        wcs_b = pool.tile(
                            [128, 2 * KC * 128], BF16, tag="wcs_b", bufs=4
                        )
                        nc.sync.dma_start(wcs_b[:], wcs[m])
                        wcat_b = wcs_b[:, 0 : KC * 128]
                        wspt2_b = wcs_b[:, KC * 128 : 2 * KC * 128]
                        u_b = pool.tile([128, 512], BF16, tag="u_b", bufs=6)
                        nc.scalar.dma_start(u_b[:], uT[m * 128 : (m + 1) * 128, nsl])
                        msl = slice(m * 128, (m + 1) * 128)
                        ps_cat = pp.tile([128, 512], F32, tag="cat", bufs=3)
                        # head contribution: expansion matmuls over A (full
                        # 512-part first: its start=True zeroes the chunk)
                        for i, (g, a, b) in enumerate(parts[n]):
                            lo = wins[g][0]
                            nc.tensor.matmul(
                                ps_cat[:, a - 512 * n : b - 512 * n],
                                A[g][:, msl],
                                E_t[:, wofs[g] + (a - lo) : wofs[g] + (b - lo)],
                                start=(i == 0),
                                stop=False,
                                skip_group_check=True,
                            )
                        # tail contribution: dense per-relation matmul
                        for k in range(KC):
                            nc.tensor.matmul(
                                ps_cat[:],
                                wcat_b[:, k * 128 : (k + 1) * 128],
                                eT[k][:, nsl],
                                start=False,
                                stop=(k == KC - 1),
                                skip_group_check=True,
                            )
                        ps_spt = pp.tile([128, 512], F32, tag="spt", bufs=2)
                        for k in range(KC):
                            nc.tensor.matmul(
                                ps_spt[:],
                                wspt2_b[:, k * 128 : (k + 1) * 128],
                                s1T[k][:, nsl],
                                start=(k == 0),
                                stop=(k == KC - 1),
                            )
                        r1 = pool.tile([128, 512], BF16, tag="r1", bufs=3)
                        nc.scalar.activation(
                            r1[:], ps_cat[:], AF.Relu, bias=bp_t[:, m : m + 1]
                        )
                        r2 = pool.tile([128, 512], BF16, tag="r2", bufs=3)
                        nc.vector.tensor_scalar(
                            out=r2[:],
                            in0=ps_spt[:],
                            scalar1=bp_t[:, MO + m : MO + m + 1],
                            scalar2=0.0,
                            op0=ALU.add,
                            op1=ALU.max,
                        )
                        pc = pool.tile([128, 512], BF16, tag="pc", bufs=4)
                        nc.vector.tensor_mul(out=pc[:], in0=r1[:], in1=r2[:])
                        lag.append((pc, u_b, m))
                    while lag and (len(lag) > 2 or m == MO):
                        pc_l, u_l, m_l = lag.pop(0)
                        nc.tensor.matmul(
                            psum_cg[:],
                            wcg_t[:, m_l, :],
                            pc_l[:],
                            start=(m_l == 0),
                            stop=False,
                            skip_group_check=True,
                        )
                        nc.tensor.matmul(
                            psum_cg[:],
                            wvis_t[:, m_l, :],
                            u_l[:],
                            start=False,
                            stop=(m_l == MO - 1),
                            skip_group_check=True,
                        )

                # -- epilogue: rel^T = (ctx + b_ctx) * sigmoid(vis+gate+frq) --
                sarg = pool.tile([128, 512], F32, tag="sarg", bufs=2)
                nc.vector.tensor_add(
                    out=sarg[GOFF : GOFF + NRC, :],
                    in0=psum_cg[GOFF : GOFF + NRC, :],
                    in1=gfT_t[GOFF : GOFF + NRC, nsl],
                )
                sg = pool.tile([128, 512], BF16, tag="sg", bufs=2)
                nc.scalar.activation(
                    sg[0:NRC, :], sarg[GOFF : GOFF + NRC, :], AF.Sigmoid
                )
                nc.vector.scalar_tensor_tensor(
                    out=outT[0:NRC, nsl],
                    in0=psum_cg[0:NRC, :],
                    scalar=bp_t[0:NRC, 2 * MO : 2 * MO + 1],
                    in1=sg[0:NRC, :],
                    op0=ALU.add,
                    op1=ALU.mult,
                )
                nc.sync.dma_start(out_t[:, nsl], outT[0:NRC, nsl])

    nc.compile()
    _nc_cache[(sl, sr)] = nc
    return nc


def _prep_core(inputs, c, common):
    perm = common["_perm"]
    sl = perm[c * NRELC : (c + 1) * NRELC]
    pi = np.asarray(inputs["pair_idx"])[sl].astype(np.int64)
    pp_ = np.asarray(inputs["pair_pred"])[sl].astype(np.int64)
    bbox = np.asarray(inputs["pair_bbox"])[sl].astype(np.float32)
    uf = np.asarray(inputs["union_features"])[sl].astype(np.float32)

    ectx = common["_ectx_bf16"]
    h = pi[:, 0]
    base = (int(h[0]) // 128) * 128
    assert int(h[-1]) < base + OBC * 128, "head span exceeds OBC chunks"

    # object slab, transposed, k-major single tile: [128, KC*OBC*128]
    eo = np.zeros((OBC * 128, H), dtype=NPBF16)
    hi_obj = min(base + OBC * 128, N_OBJ)
    eo[: hi_obj - base] = ectx[base:hi_obj]
    ectxTo = np.ascontiguousarray(
        eo.T.reshape(KC, 128, OBC * 128).transpose(1, 0, 2).reshape(128, KC * OBC * 128)
    )

    # block one-hot expansion matrix with fixed windows
    wins, wofs = common["_wins"], common["_wofs"]
    E = np.zeros((128, wofs[-1]), dtype=NPBF16)
    g_all = (h - base) // 128
    for j in range(NRELC):
        g = int(g_all[j])
        lo, hi = wins[g]
        assert lo <= j < hi, "relation outside its group's fixed window"
        E[int(h[j] - base) % 128, wofs[g] + (j - lo)] = 1.0

    e_tail = ectx[pi[:, 1]]  # [NRELC, 512]
    eTd = np.ascontiguousarray(e_tail.T).reshape(KC, 128, NRELC)

    gf = common["_freq_f32"][pp_[:, 0] * NOC + pp_[:, 1]] + common["_bvg"]
    bboxT_l = np.ones((33, NRELC), dtype=np.float32)
    bboxT_l[:32] = bbox.T
    m = {
        "eTd": eTd,
        "ectxTo": ectxTo,
        "Ed": E,
        "gfTd": np.ascontiguousarray(gf.T.astype(np.float32)),
        "bboxT": np.ascontiguousarray(bboxT_l).astype(NPBF16),
        "uT": np.ascontiguousarray(uf.T).astype(NPBF16),
    }
    m.update({k: v for k, v in common.items() if not k.startswith("_")})
    return m


def _prep_common(inputs):
    f32 = lambda k: np.asarray(inputs[k], dtype=np.float32)

    perm = np.argsort(np.asarray(inputs["pair_idx"])[:, 0], kind="stable")
    # measure per-core group-boundary deviations to size expansion windows
    heads_s = np.asarray(inputs["pair_idx"])[perm, 0]
    devs = []
    for c in range(NCORES):
        h = heads_s[c * NRELC : (c + 1) * NRELC]
        base = (int(h[0]) // 128) * 128
        g = (h - base) // 128
        for gg in range(1, OBC):
            devs.append(int(np.searchsorted(g, gg)) - 512 * gg)
    sl = max(16, ((-min(devs) + 16 + 15) // 16) * 16)
    sr = max(16, ((max(devs) + 16 + 15) // 16) * 16)

    wemb = f32("W_post_emb")  # [512, 1024]
    wcat0 = f32("W_post_cat")  # [1024, 4096]
    # fold: ctx_rep @ W_post_cat == [Eh|Et] @ [[Wh@Wcat_top];[Wt@Wcat_bot]]
    wfold_h = wemb[:, :H] @ wcat0[:H]  # [512, 4096]
    wfold_t = wemb[:, H:] @ wcat0[H:]  # [512, 4096]
    wcat_l = wfold_t.reshape(KC, 128, MO, 128).transpose(2, 1, 0, 3).reshape(
        MO, 128, KC * 128
    )
    wfh_l = np.ascontiguousarray(wfold_h.reshape(KC, 128, P)).astype(NPBF16)

    wspt2 = f32("W_spt2")  # [512, 4096]
    wspt2_l = wspt2.reshape(KC, 128, MO, 128).transpose(2, 1, 0, 3).reshape(
        MO, 128, KC * 128
    )
    wcs_l = np.ascontiguousarray(
        np.concatenate([wcat_l, wspt2_l], axis=2)
    ).astype(NPBF16)

    wcg = np.zeros((P, 128), dtype=np.float32)
    wcg[:, :NRC] = f32("W_ctx")
    wcg[:, GOFF : GOFF + NRC] = f32("W_gate")
    wcg_l = np.ascontiguousarray(
        wcg.reshape(MO, 128, 128).transpose(1, 0, 2).reshape(128, MO * 128)
    ).astype(NPBF16)

    wvis = np.zeros((P, 128), dtype=np.float32)
    wvis[:, GOFF : GOFF + NRC] = f32("W_vis")
    wvis_l = np.ascontiguousarray(
        wvis.reshape(MO, 128, 128).transpose(1, 0, 2).reshape(128, MO * 128)
    ).astype(NPBF16)

    col = lambda b, n: np.ascontiguousarray(
        np.asarray(b, dtype=np.float32).reshape(n, 128).T
    )
    bpack_l = np.zeros((128, 2 * MO + 1), dtype=np.float32)
    bpack_l[:, 0:MO] = col(
        f32("b_post_emb")[:H] @ wcat0[:H]
        + f32("b_post_emb")[H:] @ wcat0[H:]
        + f32("b_post_cat"),
        MO,
    )
    bpack_l[:, MO : 2 * MO] = col(inputs["b_spt2"], MO)
    bpack_l[:NRC, 2 * MO] = f32("b_ctx")

    wspt1_l = np.zeros((33, H), dtype=np.float32)
    wspt1_l[:32] = f32("W_spt1")
    wspt1_l[32] = f32("b_spt1")

    wins, wofs, _ = _windows(sl, sr)
    return {
        "_perm": perm,
        "_sl": sl,
        "_sr": sr,
        "_wins": wins,
        "_wofs": wofs,
        "_ectx_bf16": f32("edge_ctx").astype(NPBF16),
        "_freq_f32": f32("freq_table"),
        "_bvg": (f32("b_vis") + f32("b_gate"))[None, :],
        "wcs": wcs_l,
        "wfh": wfh_l,
        "wspt1": wspt1_l.astype(NPBF16),
        "wcg": wcg_l,
        "wvisp": wvis_l,
        "bpack": bpack_l,
    }


def kernel(**inputs) -> np.ndarray:
    global last_exec_time_ns
    trace = bool(os.environ.get("BASS_KERNEL_TRACE"))
    if trace:
        _register_ntff_hook()
    common = _prep_common(inputs)
    nc = _build(common["_sl"], common["_sr"])
    in_maps = [_prep_core(inputs, c, common) for c in range(NCORES)]
    res = run_bass_kernel_spmd(nc, in_maps, list(range(NCORES)), trace=trace)
    if trace:
        last_exec_time_ns = res.exec_time_ns
    out_sorted = np.concatenate(
        [np.asarray(res.results[c]["out_t"]).T for c in range(NCORES)], axis=0
    ).astype(np.float32)
    out = np.empty_like(out_sorted)
    out[common["_perm"]] = out_sorted
    return np.ascontiguousarray(out)


# revision 32
# speedup vs baseline: 1.0453x; 1.0206x over previous
"""Trainium2 Bass kernel for CausalAnalysisPredictor (gnn_message_passing).

kernel(**inputs) takes the FULL unsharded inputs and returns the FULL
[16384, 51] float32 output. Relations are sorted by head object on the host
and sharded contiguously across 8 NeuronCores. The head half of the folded
post_cat contraction exploits the object-level structure: per-object rows
A = edge_ctx @ Wfold_head are computed once per core (~640 objects) and
expanded to relations with block-one-hot E matmuls (fixed column windows,
host-zero-padded so the same instruction stream is valid on every core).
The tail half stays a per-relation dense matmul on host-gathered context.
"""

import os
import sys
import types

import numpy as np

try:
    import concourse  # noqa: F401
except ImportError:  # pragma: no cover
    sys.path.insert(0, "/opt/trn_rl_repo")

import ml_dtypes

import concourse.mybir as mybir
import concourse.tile as tile
from concourse import bacc
from concourse.bass_utils import run_bass_kernel_spmd

BF16 = mybir.dt.bfloat16
F32 = mybir.dt.float32
NPBF16 = ml_dtypes.bfloat16

N_OBJ, N_REL = 4096, 16384
H, P = 512, 4096
NOC, NRC = 151, 51
NCORES = 8
NRELC = N_REL // NCORES  # 2048 relations per core
KC = H // 128            # 4 feat chunks (spt1 hidden & per-side edge ctx)
MO = P // 128            # 32 output-feature chunks
NCH = NRELC // 512       # 4 relation chunks of 512
GOFF = 64                # partition offset of the gate/vis/freq lane block
OBC = 5                  # head-object 128-chunks per core (span <= 640)


def _windows(sl, sr):
    """Expansion windows [512g - sl, 512(g+1) + sr) in relation-column space.

    sl/sr bound how far any group's true column range can start before /
    end after its nominal 512-aligned slot (measured from the input on the
    host; the same values must hold on every core for the shared program).
    """
    wins = [
        (max(0, 512 * g - sl), min(NRELC, 512 * (g + 1) + sr)) for g in range(OBC)
    ]
    wofs = [0]
    for lo, hi in wins:
        wofs.append(wofs[-1] + (hi - lo))
    parts = []
    for n in range(NCH):
        c0, c1 = 512 * n, 512 * n + 512
        ps = []
        for g in range(OBC):
            lo, hi = wins[g]
            a, b = max(lo, c0), min(hi, c1)
            if a < b:
                ps.append((g, a, b))
        ps.sort(key=lambda p: -(p[2] - p[1]))  # full 512-part first (start=True)
        assert ps[0][2] - ps[0][1] == 512
        parts.append(ps)
    return wins, wofs, parts

AF = mybir.ActivationFunctionType
ALU = mybir.AluOpType

last_exec_time_ns = None  # set when BASS_KERNEL_TRACE=1


def _register_ntff_hook():
    if "antenv.axon_hooks" in sys.modules:
        return
    hook = None
    try:
        from trn_agent_boot.trn_boot import _ntff_profile_via_ctypes

        hook = _ntff_profile_via_ctypes("/opt/axon/libaxon_pjrt.so")
    except Exception:
        hook = None
    mod = types.ModuleType("antenv.axon_hooks")
    mod.get_axon_ntff_profile_hook = lambda: hook
    mod.set_axon_ntff_profile_hook = lambda h: None
    sys.modules["antenv.axon_hooks"] = mod


_nc_cache = {}


def _build(sl, sr):
    if (sl, sr) in _nc_cache:
        return _nc_cache[(sl, sr)]
    wins, wofs, parts = _windows(sl, sr)
    ecols = wofs[-1]

    nc = bacc.Bacc("TRN2", target_bir_lowering=False, debug=False, num_devices=NCORES)

    # ---- DRAM parameters (per-core shards / replicated tables) ----
    eTd = nc.declare_dram_parameter("eTd", [KC, 128, NRELC], BF16, isOutput=False)
    # one [128, KC*OBC*128] tile (k-major columns) -> 128 large descriptors
    ectxTo = nc.declare_dram_parameter("ectxTo", [128, KC * OBC * 128], BF16, isOutput=False)
    wfh = nc.declare_dram_parameter("wfh", [8, 128, KC * 512], BF16, isOutput=False)
    Ed = nc.declare_dram_parameter("Ed", [128, ecols], BF16, isOutput=False)
    gfTd = nc.declare_dram_parameter("gfTd", [NRC, NRELC], F32, isOutput=False)
    # row 32 of bboxT is all-ones and row 32 of wspt1 is b_spt1 (bias fold)
    bboxT = nc.declare_dram_parameter("bboxT", [33, NRELC], BF16, isOutput=False)
    uT = nc.declare_dram_parameter("uT", [P, NRELC], BF16, isOutput=False)
    # per-m merged stream: [tail wcat | wspt2] -> one DMA per (n, m)
    wcs = nc.declare_dram_parameter("wcs", [MO, 128, 2 * KC * 128], BF16, isOutput=False)
    wspt1 = nc.declare_dram_parameter("wspt1", [33, H], BF16, isOutput=False)
    wcg = nc.declare_dram_parameter("wcg", [128, MO * 128], BF16, isOutput=False)
    wvisp = nc.declare_dram_parameter("wvisp", [128, MO * 128], BF16, isOutput=False)
    # packed biases: cols [0:32]=bcat, [32:64]=bs2, [64]=bctx
    bpack = nc.declare_dram_parameter("bpack", [128, 2 * MO + 1], F32, isOutput=False)
    out_t = nc.declare_dram_parameter("out_t", [NRC, NRELC], F32, isOutput=True)

    with tile.TileContext(nc) as tc:
        with (
            tc.tile_pool(name="sbuf", bufs=1) as pool,
            tc.tile_pool(name="psum", bufs=1, space="PSUM") as pp,
        ):
            # ---- phase-0 loads: spt1 + A-phase inputs lead their queues ----
            wspt1_t = pool.tile([33, H], BF16)
            nc.scalar.dma_start(wspt1_t[:], wspt1[:])
            bboxT_t = pool.tile([33, NRELC], BF16)
            nc.scalar.dma_start(bboxT_t[:], bboxT[:])
            ectxTo_t = pool.tile([128, KC * OBC * 128], BF16)
            nc.sync.dma_start(ectxTo_t[:], ectxTo[:])
            bp_t = pool.tile([128, 2 * MO + 1], F32)
            nc.scalar.dma_start(bp_t[:], bpack[:])
            # wfh: fs-major blocks, one 128-descriptor DMA each; fs=0 on sync
            # unblocks the A phase, the rest ride the scalar queue
            wfh_t = [pool.tile([128, KC * 512], BF16, name=f"wfh{f}") for f in range(8)]
            nc.sync.dma_start(wfh_t[0][:], wfh[0])
            for f in range(1, 8):
                nc.scalar.dma_start(wfh_t[f][:], wfh[f])
            E_t = pool.tile([128, ecols], BF16)
            nc.gpsimd.dma_start(E_t[:], Ed[:])
            eT = [pool.tile([128, NRELC], BF16, name=f"eT{k}") for k in range(KC)]
            for k in range(KC):
                eng = nc.scalar if k < 2 else nc.gpsimd
                eng.dma_start(eT[k][:], eTd[k])
            wcg_t = pool.tile([128, MO, 128], BF16)
            nc.gpsimd.dma_start(wcg_t[:], wcg[:].rearrange("p (m c) -> p m c", m=MO))
            wvis_t = pool.tile([128, MO, 128], BF16)
            nc.gpsimd.dma_start(wvis_t[:], wvisp[:].rearrange("p (m c) -> p m c", m=MO))
            gfT_t = pool.tile([128, NRELC], F32)
            nc.gpsimd.dma_start(gfT_t[GOFF : GOFF + NRC, :], gfTd[:])

            # ---- spt1 (bbox only; warms the PE while DMAs stream) ----
            s1T = [pool.tile([128, NRELC], BF16, name=f"s1T{k}") for k in range(KC)]
            for k in range(KC):
                for n in range(NCH):
                    ps = pp.tile([128, 512], F32, tag="cat", bufs=3)
                    nc.tensor.matmul(
                        ps[:],
                        wspt1_t[:, k * 128 : (k + 1) * 128],
                        bboxT_t[:, n * 512 : (n + 1) * 512],
                        start=True,
                        stop=True,
                    )
                    nc.scalar.activation(
                        s1T[k][:, n * 512 : (n + 1) * 512], ps[:], AF.Relu
                    )

            # ---- A phase: per-object head reps A[g] = ectx_chunk @ Wfold_h ----
            # fs-outer so each freshly-arrived wfh column block feeds OBC
            # matmul groups before the next block is needed (DMA pipelining)
            A = [pool.tile([128, P], BF16, name=f"A{g}") for g in range(OBC)]
            for fs in range(8):
                fsl = slice(fs * 512, (fs + 1) * 512)
                for g in range(OBC):
                    ps = pp.tile([128, 512], F32, tag="cat", bufs=3)
                    for k in range(KC):
                        osl = slice(k * OBC * 128 + g * 128, k * OBC * 128 + (g + 1) * 128)
                        nc.tensor.matmul(
                            ps[:],
                            ectxTo_t[:, osl],
                            wfh_t[fs][:, k * 512 : (k + 1) * 512],
                            start=(k == 0),
                            stop=(k == KC - 1),
                        )
                    nc.scalar.activation(A[g][:, fsl], ps[:], AF.Copy)

            outT = pool.tile([128, NRELC], F32)

            for n in range(NCH):
                nsl = slice(n * 512, (n + 1) * 512)
                psum_cg = pp.tile([128, 512], F32, tag="cg", bufs=2)
                lag = []  # (pc, u_b, m) awaiting their cg/vis matmuls
                for m in range(MO + 1):
                    if m < MO:
                        wcs_b = pool.tile(
                            [128, 2 * KC * 128], BF16, tag="wcs_b", bufs=4
                        )
                        nc.sync.dma_start(wcs_b[:], wcs[m])
                        wcat_b = wcs_b[:, 0 : KC * 128]
                        wspt2_b = wcs_b[:, KC * 128 : 2 * KC * 128]
                        u_b = pool.tile([128, 512], BF16, tag="u_b", bufs=6)
                        nc.scalar.dma_start(u_b[:], uT[m * 128 : (m + 1) * 128, nsl])
                        msl = slice(m * 128, (m + 1) * 128)
                        ps_cat = pp.tile([128, 512], F32, tag="cat", bufs=3)
                        # head contribution: expansion matmuls over A (full
                        # 512-part first: its start=True zeroes the chunk)
                        for i, (g, a, b) in enumerate(parts[n]):
                            lo = wins[g][0]
                            nc.tensor.matmul(
                                ps_cat[:, a - 512 * n : b - 512 * n],
                                A[g][:, msl],
                                E_t[:, wofs[g] + (a - lo) : wofs[g] + (b - lo)],
                                start=(i == 0),
                                stop=False,
                                skip_group_check=True,
                            )
                        # tail contribution: dense per-relation matmul
                        for k in range(KC):
                            nc.tensor.matmul(
                                ps_cat[:],
                                wcat_b[:, k * 128 : (k + 1) * 128],
                                eT[k][:, nsl],
                                start=False,
                                stop=(k == KC - 1),
                                skip_group_check=True,
                            )
                        ps_spt = pp.tile([128, 512], F32, tag="spt", bufs=2)
                        for k in range(KC):
                            nc.tensor.matmul(
                                ps_spt[:],
                                wspt2_b[:, k * 128 : (k + 1) * 128],
                                s1T[k][:, nsl],
                                start=(k == 0),
                                stop=(k == KC - 1),
                            )
                        r1 = pool.tile([128, 512], BF16, tag="r1", bufs=3)
                        nc.scalar.activation(
                            r1[:], ps_cat[:], AF.Relu, bias=bp_t[:, m : m + 1]
                        )
                        r2 = pool.tile([128, 512], BF16, tag="r2", bufs=3)
                        nc.vector.tensor_scalar(
                            out=r2[:],
                            in0=ps_spt[:],
                            scalar1=bp_t[:, MO + m : MO + m + 1],
                            scalar2=0.0,
                            op0=ALU.add,
                            op1=ALU.max,
                        )
                        pc = pool.tile([128, 512], BF16, tag="pc", bufs=4)
                        nc.vector.tensor_mul(out=pc[:], in0=r1[:], in1=r2[:])
                        lag.append((pc, u_b, m))
                    while lag and (len(lag) > 2 or m == MO):
                        pc_l, u_l, m_l = lag.pop(0)
                        nc.tensor.matmul(
                            psum_cg[:],
                            wcg_t[:, m_l, :],
                            pc_l[:],
                            start=(m_l == 0),
                            stop=False,
                            skip_group_check=True,
                        )
                        nc.tensor.matmul(
                            psum_cg[:],
                            wvis_t[:, m_l, :],
                            u_l[:],
                            start=False,
                            stop=(m_l == MO - 1),
                            skip_group_check=True,
                        )

                # -- epilogue: rel^T = (ctx + b_ctx) * sigmoid(vis+gate+frq) --
                # last chunk runs in two halves so the final drain is shorter
                halves = (
                    [(0, 512)] if n < NCH - 1 else [(0, 256), (256, 512)]
                )
                for h0, h1 in halves:
                    hw_ = h1 - h0
                    hsl = slice(n * 512 + h0, n * 512 + h1)
                    sarg = pool.tile([128, 512], F32, tag="sarg", bufs=2)
                    nc.vector.tensor_add(
                        out=sarg[GOFF : GOFF + NRC, 0:hw_],
                        in0=psum_cg[GOFF : GOFF + NRC, h0:h1],
                        in1=gfT_t[GOFF : GOFF + NRC, hsl],
                    )
                    sg = pool.tile([128, 512], BF16, tag="sg", bufs=2)
                    nc.scalar.activation(
                        sg[0:NRC, 0:hw_], sarg[GOFF : GOFF + NRC, 0:hw_], AF.Sigmoid
                    )
                    nc.vector.scalar_tensor_tensor(
                        out=outT[0:NRC, hsl],
                        in0=psum_cg[0:NRC, h0:h1],
                        scalar=bp_t[0:NRC, 2 * MO : 2 * MO + 1],
                        in1=sg[0:NRC, 0:hw_],
                        op0=ALU.add,
                        op1=ALU.mult,
                    )
                    nc.sync.dma_start(out_t[:, hsl], outT[0:NRC, hsl])

    nc.compile()
    _nc_cache[(sl, sr)] = nc
    return nc


def _prep_core(inputs, c, common):
    perm = common["_perm"]
    sl = perm[c * NRELC : (c + 1) * NRELC]
    pi = np.asarray(inputs["pair_idx"])[sl].astype(np.int64)
    pp_ = np.asarray(inputs["pair_pred"])[sl].astype(np.int64)
    bbox = np.asarray(inputs["pair_bbox"])[sl].astype(np.float32)
    uf = np.asarray(inputs["union_features"])[sl].astype(np.float32)

    ectx = common["_ectx_bf16"]
    h = pi[:, 0]
    base = (int(h[0]) // 128) * 128
    assert int(h[-1]) < base + OBC * 128, "head span exceeds OBC chunks"

    # object slab, transposed, k-major single tile: [128, KC*OBC*128]
    eo = np.zeros((OBC * 128, H), dtype=NPBF16)
    hi_obj = min(base + OBC * 128, N_OBJ)
    eo[: hi_obj - base] = ectx[base:hi_obj]
    ectxTo = np.ascontiguousarray(
        eo.T.reshape(KC, 128, OBC * 128).transpose(1, 0, 2).reshape(128, KC * OBC * 128)
    )

    # block one-hot expansion matrix with fixed windows
    wins, wofs = common["_wins"], common["_wofs"]
    E = np.zeros((128, wofs[-1]), dtype=NPBF16)
    g_all = (h - base) // 128
    for j in range(NRELC):
        g = int(g_all[j])
        lo, hi = wins[g]
        assert lo <= j < hi, "relation outside its group's fixed window"
        E[int(h[j] - base) % 128, wofs[g] + (j - lo)] = 1.0

    e_tail = ectx[pi[:, 1]]  # [NRELC, 512]
    eTd = np.ascontiguousarray(e_tail.T).reshape(KC, 128, NRELC)

    gf = common["_freq_f32"][pp_[:, 0] * NOC + pp_[:, 1]] + common["_bvg"]
    bboxT_l = np.ones((33, NRELC), dtype=np.float32)
    bboxT_l[:32] = bbox.T
    m = {
        "eTd": eTd,
        "ectxTo": ectxTo,
        "Ed": E,
        "gfTd": np.ascontiguousarray(gf.T.astype(np.float32)),
        "bboxT": np.ascontiguousarray(bboxT_l).astype(NPBF16),
        "uT": np.ascontiguousarray(uf.T).astype(NPBF16),
    }
    m.update({k: v for k, v in common.items() if not k.startswith("_")})
    return m


def _prep_common(inputs):
    f32 = lambda k: np.asarray(inputs[k], dtype=np.float32)

    perm = np.argsort(np.asarray(inputs["pair_idx"])[:, 0], kind="stable")
    # measure per-core group-boundary deviations to size expansion windows
    heads_s = np.asarray(inputs["pair_idx"])[perm, 0]
    devs = []
    for c in range(NCORES):
        h = heads_s[c * NRELC : (c + 1) * NRELC]
        base = (int(h[0]) // 128) * 128
        g = (h - base) // 128
        for gg in range(1, OBC):
            devs.append(int(np.searchsorted(g, gg)) - 512 * gg)
    sl = max(16, ((-min(devs) + 16 + 15) // 16) * 16)
    sr = max(16, ((max(devs) + 16 + 15) // 16) * 16)

    wemb = f32("W_post_emb")  # [512, 1024]
    wcat0 = f32("W_post_cat")  # [1024, 4096]
    # fold: ctx_rep @ W_post_cat == [Eh|Et] @ [[Wh@Wcat_top];[Wt@Wcat_bot]]
    wfold_h = wemb[:, :H] @ wcat0[:H]  # [512, 4096]
    wfold_t = wemb[:, H:] @ wcat0[H:]  # [512, 4096]
    wcat_l = wfold_t.reshape(KC, 128, MO, 128).transpose(2, 1, 0, 3).reshape(
        MO, 128, KC * 128
    )
    wfh_l = np.ascontiguousarray(
        wfold_h.reshape(KC, 128, 8, 512).transpose(2, 1, 0, 3).reshape(8, 128, KC * 512)
    ).astype(NPBF16)

    wspt2 = f32("W_spt2")  # [512, 4096]
    wspt2_l = wspt2.reshape(KC, 128, MO, 128).transpose(2, 1, 0, 3).reshape(
        MO, 128, KC * 128
    )
    wcs_l = np.ascontiguousarray(
        np.concatenate([wcat_l, wspt2_l], axis=2)
    ).astype(NPBF16)

    wcg = np.zeros((P, 128), dtype=np.float32)
    wcg[:, :NRC] = f32("W_ctx")
    wcg[:, GOFF : GOFF + NRC] = f32("W_gate")
    wcg_l = np.ascontiguousarray(
        wcg.reshape(MO, 128, 128).transpose(1, 0, 2).reshape(128, MO * 128)
    ).astype(NPBF16)

    wvis = np.zeros((P, 128), dtype=np.float32)
    wvis[:, GOFF : GOFF + NRC] = f32("W_vis")
    wvis_l = np.ascontiguousarray(
        wvis.reshape(MO, 128, 128).transpose(1, 0, 2).reshape(128, MO * 128)
    ).astype(NPBF16)

    col = lambda b, n: np.ascontiguousarray(
        np.asarray(b, dtype=np.float32).reshape(n, 128).T
    )
    bpack_l = np.zeros((128, 2 * MO + 1), dtype=np.float32)
    bpack_l[:, 0:MO] = col(
        f32("b_post_emb")[:H] @ wcat0[:H]
        + f32("b_post_emb")[H:] @ wcat0[H:]
        + f32("b_post_cat"),
        MO,
    )
    bpack_l[:, MO : 2 * MO] = col(inputs["b_spt2"], MO)
    bpack_l[:NRC, 2 * MO] = f32("b_ctx")

    wspt1_l = np.zeros((33, H), dtype=np.float32)
    wspt1_l[:32] = f32("W_spt1")
    wspt1_l[32] = f32("b_spt1")

    wins, wofs, _ = _windows(sl, sr)
    return {
        "_perm": perm,
        "_sl": sl,
        "_sr": sr,
        "_wins": wins,
        "_wofs": wofs,
        "_ectx_bf16": f32("edge_ctx").astype(NPBF16),
        "_freq_f32": f32("freq_table"),
        "_bvg": (f32("b_vis") + f32("b_gate"))[None, :],
        "wcs": wcs_l,
        "wfh": wfh_l,
        "wspt1": wspt1_l.astype(NPBF16),
        "wcg": wcg_l,
        "wvisp": wvis_l,
        "bpack": bpack_l,
    }


def kernel(**inputs) -> np.ndarray:
    global last_exec_time_ns
    trace = bool(os.environ.get("BASS_KERNEL_TRACE"))
    if trace:
        _register_ntff_hook()
    common = _prep_common(inputs)
    nc = _build(common["_sl"], common["_sr"])
    in_maps = [_prep_core(inputs, c, common) for c in range(NCORES)]
    res = run_bass_kernel_spmd(nc, in_maps, list(range(NCORES)), trace=trace)
    if trace:
        last_exec_time_ns = res.exec_time_ns
    out_sorted = np.concatenate(
        [np.asarray(res.results[c]["out_t"]).T for c in range(NCORES)], axis=0
    ).astype(np.float32)
    out = np.empty_like(out_sorted)
    out[common["_perm"]] = out_sorted
    return np.ascontiguousarray(out)


# revision 33
# speedup vs baseline: 1.0855x; 1.0385x over previous
"""Trainium2 Bass kernel for CausalAnalysisPredictor (gnn_message_passing).

kernel(**inputs) takes the FULL unsharded inputs and returns the FULL
[16384, 51] float32 output. Relations are sorted by head object on the host
and sharded contiguously across 8 NeuronCores. The head half of the folded
post_cat contraction exploits the object-level structure: per-object rows
A = edge_ctx @ Wfold_head are computed once per core (~640 objects) and
expanded to relations with block-one-hot E matmuls (fixed column windows,
host-zero-padded so the same instruction stream is valid on every core).
The tail half stays a per-relation dense matmul on host-gathered context.
"""

import os
import sys
import types

import numpy as np

try:
    import concourse  # noqa: F401
except ImportError:  # pragma: no cover
    sys.path.insert(0, "/opt/trn_rl_repo")

import ml_dtypes

import concourse.mybir as mybir
import concourse.tile as tile
from concourse import bacc
from concourse.bass_utils import run_bass_kernel_spmd

BF16 = mybir.dt.bfloat16
F32 = mybir.dt.float32
NPBF16 = ml_dtypes.bfloat16

N_OBJ, N_REL = 4096, 16384
H, P = 512, 4096
NOC, NRC = 151, 51
NCORES = 8
NRELC = N_REL // NCORES  # 2048 relations per core
KC = H // 128            # 4 feat chunks (spt1 hidden & per-side edge ctx)
MO = P // 128            # 32 output-feature chunks
NCH = NRELC // 512       # 4 relation chunks of 512
GOFF = 64                # partition offset of the gate/vis/freq lane block
OBC = 5                  # head-object 128-chunks per core (span <= 640)


def _windows(sl, sr):
    """Expansion windows [512g - sl, 512(g+1) + sr) in relation-column space.

    sl/sr bound how far any group's true column range can start before /
    end after its nominal 512-aligned slot (measured from the input on the
    host; the same values must hold on every core for the shared program).
    """
    wins = [
        (max(0, 512 * g - sl), min(NRELC, 512 * (g + 1) + sr)) for g in range(OBC)
    ]
    wofs = [0]
    for lo, hi in wins:
        wofs.append(wofs[-1] + (hi - lo))
    parts = []
    for n in range(NCH):
        c0, c1 = 512 * n, 512 * n + 512
        ps = []
        for g in range(OBC):
            lo, hi = wins[g]
            a, b = max(lo, c0), min(hi, c1)
            if a < b:
                ps.append((g, a, b))
        ps.sort(key=lambda p: -(p[2] - p[1]))  # full 512-part first (start=True)
        assert ps[0][2] - ps[0][1] == 512
        parts.append(ps)
    return wins, wofs, parts

AF = mybir.ActivationFunctionType
ALU = mybir.AluOpType

last_exec_time_ns = None  # set when BASS_KERNEL_TRACE=1


def _register_ntff_hook():
    if "antenv.axon_hooks" in sys.modules:
        return
    hook = None
    try:
        from trn_agent_boot.trn_boot import _ntff_profile_via_ctypes

        hook = _ntff_profile_via_ctypes("/opt/axon/libaxon_pjrt.so")
    except Exception:
        hook = None
    mod = types.ModuleType("antenv.axon_hooks")
    mod.get_axon_ntff_profile_hook = lambda: hook
    mod.set_axon_ntff_profile_hook = lambda h: None
    sys.modules["antenv.axon_hooks"] = mod


_nc_cache = {}


def _build(sl, sr):
    if (sl, sr) in _nc_cache:
        return _nc_cache[(sl, sr)]
    wins, wofs, parts = _windows(sl, sr)
    ecols = wofs[-1]

    nc = bacc.Bacc("TRN2", target_bir_lowering=False, debug=False, num_devices=NCORES)

    # ---- DRAM parameters (per-core shards / replicated tables) ----
    eTd = nc.declare_dram_parameter("eTd", [KC, 128, NRELC], BF16, isOutput=False)
    # one [128, KC*OBC*128] tile (k-major columns) -> 128 large descriptors
    ectxTo = nc.declare_dram_parameter("ectxTo", [128, KC * OBC * 128], BF16, isOutput=False)
    wfh = nc.declare_dram_parameter("wfh", [8, 128, KC * 512], BF16, isOutput=False)
    Ed = nc.declare_dram_parameter("Ed", [128, ecols], BF16, isOutput=False)
    gfTd = nc.declare_dram_parameter("gfTd", [NRC, NRELC], F32, isOutput=False)
    # row 32 of bboxT is all-ones and row 32 of wspt1 is b_spt1 (bias fold)
    bboxT = nc.declare_dram_parameter("bboxT", [33, NRELC], BF16, isOutput=False)
    uT = nc.declare_dram_parameter("uT", [P, NRELC], BF16, isOutput=False)
    # per-m merged stream: [tail wcat | wspt2] -> one DMA per (n, m)
    wcs = nc.declare_dram_parameter("wcs", [MO, 128, 2 * KC * 128], BF16, isOutput=False)
    wspt1 = nc.declare_dram_parameter("wspt1", [33, H], BF16, isOutput=False)
    wcg = nc.declare_dram_parameter("wcg", [128, MO * 128], BF16, isOutput=False)
    wvisp = nc.declare_dram_parameter("wvisp", [128, MO * 128], BF16, isOutput=False)
    # packed biases: cols [0:32]=bcat, [32:64]=bs2, [64]=bctx
    bpack = nc.declare_dram_parameter("bpack", [128, 2 * MO + 1], F32, isOutput=False)
    out_t = nc.declare_dram_parameter("out_t", [NRC, NRELC], F32, isOutput=True)

    with tile.TileContext(nc) as tc:
        with (
            tc.tile_pool(name="sbuf", bufs=1) as pool,
            tc.tile_pool(name="psum", bufs=1, space="PSUM") as pp,
        ):
            # ---- phase-0 loads: spt1 + A-phase inputs lead their queues ----
            wspt1_t = pool.tile([33, H], BF16)
            nc.scalar.dma_start(wspt1_t[:], wspt1[:])
            bboxT_t = pool.tile([33, NRELC], BF16)
            nc.scalar.dma_start(bboxT_t[:], bboxT[:])
            ectxTo_t = pool.tile([128, KC * OBC * 128], BF16)
            nc.sync.dma_start(ectxTo_t[:], ectxTo[:])
            bp_t = pool.tile([128, 2 * MO + 1], F32)
            nc.gpsimd.dma_start(bp_t[:], bpack[:])
            # wfh: fs-major blocks, one 128-descriptor DMA each; fs=0 on sync
            # unblocks the A phase, the rest ride the scalar queue
            wfh_t = [pool.tile([128, KC * 512], BF16, name=f"wfh{f}") for f in range(8)]
            for f in range(8):
                nc.sync.dma_start(wfh_t[f][:], wfh[f])
            E_t = pool.tile([128, ecols], BF16)
            nc.gpsimd.dma_start(E_t[:], Ed[:])
            eT = [pool.tile([128, NRELC], BF16, name=f"eT{k}") for k in range(KC)]
            for k in range(KC):
                nc.gpsimd.dma_start(eT[k][:], eTd[k])
            wcg_t = pool.tile([128, MO, 128], BF16)
            nc.gpsimd.dma_start(wcg_t[:], wcg[:].rearrange("p (m c) -> p m c", m=MO))
            wvis_t = pool.tile([128, MO, 128], BF16)
            nc.gpsimd.dma_start(wvis_t[:], wvisp[:].rearrange("p (m c) -> p m c", m=MO))
            gfT_t = pool.tile([128, NRELC], F32)
            nc.gpsimd.dma_start(gfT_t[GOFF : GOFF + NRC, :], gfTd[:])

            # ---- spt1 (bbox only; warms the PE while DMAs stream) ----
            s1T = [pool.tile([128, NRELC], BF16, name=f"s1T{k}") for k in range(KC)]
            for k in range(KC):
                for n in range(NCH):
                    ps = pp.tile([128, 512], F32, tag="cat", bufs=3)
                    nc.tensor.matmul(
                        ps[:],
                        wspt1_t[:, k * 128 : (k + 1) * 128],
                        bboxT_t[:, n * 512 : (n + 1) * 512],
                        start=True,
                        stop=True,
                    )
                    nc.scalar.activation(
                        s1T[k][:, n * 512 : (n + 1) * 512], ps[:], AF.Relu
                    )

            # ---- A phase: per-object head reps A[g] = ectx_chunk @ Wfold_h ----
            # fs-outer so each freshly-arrived wfh column block feeds OBC
            # matmul groups before the next block is needed (DMA pipelining)
            A = [pool.tile([128, P], BF16, name=f"A{g}") for g in range(OBC)]
            for fs in range(8):
                fsl = slice(fs * 512, (fs + 1) * 512)
                for g in range(OBC):
                    ps = pp.tile([128, 512], F32, tag="cat", bufs=3)
                    for k in range(KC):
                        osl = slice(k * OBC * 128 + g * 128, k * OBC * 128 + (g + 1) * 128)
                        nc.tensor.matmul(
                            ps[:],
                            ectxTo_t[:, osl],
                            wfh_t[fs][:, k * 512 : (k + 1) * 512],
                            start=(k == 0),
                            stop=(k == KC - 1),
                        )
                    nc.scalar.activation(A[g][:, fsl], ps[:], AF.Copy)

            outT = pool.tile([128, NRELC], F32)

            for n in range(NCH):
                nsl = slice(n * 512, (n + 1) * 512)
                psum_cg = pp.tile([128, 512], F32, tag="cg", bufs=2)
                lag = []  # (pc, u_b, m) awaiting their cg/vis matmuls
                for m in range(MO + 1):
                    if m < MO:
                        wcs_b = pool.tile(
                            [128, 2 * KC * 128], BF16, tag="wcs_b", bufs=4
                        )
                        nc.sync.dma_start(wcs_b[:], wcs[m])
                        wcat_b = wcs_b[:, 0 : KC * 128]
                        wspt2_b = wcs_b[:, KC * 128 : 2 * KC * 128]
                        u_b = pool.tile([128, 512], BF16, tag="u_b", bufs=6)
                        nc.scalar.dma_start(u_b[:], uT[m * 128 : (m + 1) * 128, nsl])
                        msl = slice(m * 128, (m + 1) * 128)
                        ps_cat = pp.tile([128, 512], F32, tag="cat", bufs=3)
                        # head contribution: expansion matmuls over A (full
                        # 512-part first: its start=True zeroes the chunk)
                        for i, (g, a, b) in enumerate(parts[n]):
                            lo = wins[g][0]
                            nc.tensor.matmul(
                                ps_cat[:, a - 512 * n : b - 512 * n],
                                A[g][:, msl],
                                E_t[:, wofs[g] + (a - lo) : wofs[g] + (b - lo)],
                                start=(i == 0),
                                stop=False,
                                skip_group_check=True,
                            )
                        # tail contribution: dense per-relation matmul
                        for k in range(KC):
                            nc.tensor.matmul(
                                ps_cat[:],
                                wcat_b[:, k * 128 : (k + 1) * 128],
                                eT[k][:, nsl],
                                start=False,
                                stop=(k == KC - 1),
                                skip_group_check=True,
                            )
                        ps_spt = pp.tile([128, 512], F32, tag="spt", bufs=2)
                        for k in range(KC):
                            nc.tensor.matmul(
                                ps_spt[:],
                                wspt2_b[:, k * 128 : (k + 1) * 128],
                                s1T[k][:, nsl],
                                start=(k == 0),
                                stop=(k == KC - 1),
                            )
                        r1 = pool.tile([128, 512], BF16, tag="r1", bufs=3)
                        nc.scalar.activation(
                            r1[:], ps_cat[:], AF.Relu, bias=bp_t[:, m : m + 1]
                        )
                        r2 = pool.tile([128, 512], BF16, tag="r2", bufs=3)
                        nc.vector.tensor_scalar(
                            out=r2[:],
                            in0=ps_spt[:],
                            scalar1=bp_t[:, MO + m : MO + m + 1],
                            scalar2=0.0,
                            op0=ALU.add,
                            op1=ALU.max,
                        )
                        pc = pool.tile([128, 512], BF16, tag="pc", bufs=4)
                        nc.vector.tensor_mul(out=pc[:], in0=r1[:], in1=r2[:])
                        lag.append((pc, u_b, m))
                    while lag and (len(lag) > 2 or m == MO):
                        pc_l, u_l, m_l = lag.pop(0)
                        nc.tensor.matmul(
                            psum_cg[:],
                            wcg_t[:, m_l, :],
                            pc_l[:],
                            start=(m_l == 0),
                            stop=False,
                            skip_group_check=True,
                        )
                        nc.tensor.matmul(
                            psum_cg[:],
                            wvis_t[:, m_l, :],
                            u_l[:],
                            start=False,
                            stop=(m_l == MO - 1),
                            skip_group_check=True,
                        )

                # -- epilogue: rel^T = (ctx + b_ctx) * sigmoid(vis+gate+frq) --
                # last chunk runs in two halves so the final drain is shorter
                halves = (
                    [(0, 512)] if n < NCH - 1 else [(0, 256), (256, 512)]
                )
                for h0, h1 in halves:
                    hw_ = h1 - h0
                    hsl = slice(n * 512 + h0, n * 512 + h1)
                    sarg = pool.tile([128, 512], F32, tag="sarg", bufs=2)
                    nc.vector.tensor_add(
                        out=sarg[GOFF : GOFF + NRC, 0:hw_],
                        in0=psum_cg[GOFF : GOFF + NRC, h0:h1],
                        in1=gfT_t[GOFF : GOFF + NRC, hsl],
                    )
                    sg = pool.tile([128, 512], BF16, tag="sg", bufs=2)
                    nc.scalar.activation(
                        sg[0:NRC, 0:hw_], sarg[GOFF : GOFF + NRC, 0:hw_], AF.Sigmoid
                    )
                    nc.vector.scalar_tensor_tensor(
                        out=outT[0:NRC, hsl],
                        in0=psum_cg[0:NRC, h0:h1],
                        scalar=bp_t[0:NRC, 2 * MO : 2 * MO + 1],
                        in1=sg[0:NRC, 0:hw_],
                        op0=ALU.add,
                        op1=ALU.mult,
                    )
                    nc.sync.dma_start(out_t[:, hsl], outT[0:NRC, hsl])

    nc.compile()
    _nc_cache[(sl, sr)] = nc
    return nc


def _prep_core(inputs, c, common):
    perm = common["_perm"]
    sl = perm[c * NRELC : (c + 1) * NRELC]
    pi = np.asarray(inputs["pair_idx"])[sl].astype(np.int64)
    pp_ = np.asarray(inputs["pair_pred"])[sl].astype(np.int64)
    bbox = np.asarray(inputs["pair_bbox"])[sl].astype(np.float32)
    uf = np.asarray(inputs["union_features"])[sl].astype(np.float32)

    ectx = common["_ectx_bf16"]
    h = pi[:, 0]
    base = (int(h[0]) // 128) * 128
    assert int(h[-1]) < base + OBC * 128, "head span exceeds OBC chunks"

    # object slab, transposed, k-major single tile: [128, KC*OBC*128]
    eo = np.zeros((OBC * 128, H), dtype=NPBF16)
    hi_obj = min(base + OBC * 128, N_OBJ)
    eo[: hi_obj - base] = ectx[base:hi_obj]
    ectxTo = np.ascontiguousarray(
        eo.T.reshape(KC, 128, OBC * 128).transpose(1, 0, 2).reshape(128, KC * OBC * 128)
    )

    # block one-hot expansion matrix with fixed windows
    wins, wofs = common["_wins"], common["_wofs"]
    E = np.zeros((128, wofs[-1]), dtype=NPBF16)
    g_all = (h - base) // 128
    for j in range(NRELC):
        g = int(g_all[j])
        lo, hi = wins[g]
        assert lo <= j < hi, "relation outside its group's fixed window"
        E[int(h[j] - base) % 128, wofs[g] + (j - lo)] = 1.0

    e_tail = ectx[pi[:, 1]]  # [NRELC, 512]
    eTd = np.ascontiguousarray(e_tail.T).reshape(KC, 128, NRELC)

    gf = common["_freq_f32"][pp_[:, 0] * NOC + pp_[:, 1]] + common["_bvg"]
    bboxT_l = np.ones((33, NRELC), dtype=np.float32)
    bboxT_l[:32] = bbox.T
    m = {
        "eTd": eTd,
        "ectxTo": ectxTo,
        "Ed": E,
        "gfTd": np.ascontiguousarray(gf.T.astype(np.float32)),
        "bboxT": np.ascontiguousarray(bboxT_l).astype(NPBF16),
        "uT": np.ascontiguousarray(uf.T).astype(NPBF16),
    }
    m.update({k: v for k, v in common.items() if not k.startswith("_")})
    return m


def _prep_common(inputs):
    f32 = lambda k: np.asarray(inputs[k], dtype=np.float32)

    perm = np.argsort(np.asarray(inputs["pair_idx"])[:, 0], kind="stable")
    # measure per-core group-boundary deviations to size expansion windows
    heads_s = np.asarray(inputs["pair_idx"])[perm, 0]
    devs = []
    for c in range(NCORES):
        h = heads_s[c * NRELC : (c + 1) * NRELC]
        base = (int(h[0]) // 128) * 128
        g = (h - base) // 128
        for gg in range(1, OBC):
            devs.append(int(np.searchsorted(g, gg)) - 512 * gg)
    sl = max(16, ((-min(devs) + 16 + 15) // 16) * 16)
    sr = max(16, ((max(devs) + 16 + 15) // 16) * 16)

    wemb = f32("W_post_emb")  # [512, 1024]
    wcat0 = f32("W_post_cat")  # [1024, 4096]
    # fold: ctx_rep @ W_post_cat == [Eh|Et] @ [[Wh@Wcat_top];[Wt@Wcat_bot]]
    wfold_h = wemb[:, :H] @ wcat0[:H]  # [512, 4096]
    wfold_t = wemb[:, H:] @ wcat0[H:]  # [512, 4096]
    wcat_l = wfold_t.reshape(KC, 128, MO, 128).transpose(2, 1, 0, 3).reshape(
        MO, 128, KC * 128
    )
    wfh_l = np.ascontiguousarray(
        wfold_h.reshape(KC, 128, 8, 512).transpose(2, 1, 0, 3).reshape(8, 128, KC * 512)
    ).astype(NPBF16)

    wspt2 = f32("W_spt2")  # [512, 4096]
    wspt2_l = wspt2.reshape(KC, 128, MO, 128).transpose(2, 1, 0, 3).reshape(
        MO, 128, KC * 128
    )
    wcs_l = np.ascontiguousarray(
        np.concatenate([wcat_l, wspt2_l], axis=2)
    ).astype(NPBF16)

    wcg = np.zeros((P, 128), dtype=np.float32)
    wcg[:, :NRC] = f32("W_ctx")
    wcg[:, GOFF : GOFF + NRC] = f32("W_gate")
    wcg_l = np.ascontiguousarray(
        wcg.reshape(MO, 128, 128).transpose(1, 0, 2).reshape(128, MO * 128)
    ).astype(NPBF16)

    wvis = np.zeros((P, 128), dtype=np.float32)
    wvis[:, GOFF : GOFF + NRC] = f32("W_vis")
    wvis_l = np.ascontiguousarray(
        wvis.reshape(MO, 128, 128).transpose(1, 0, 2).reshape(128, MO * 128)
    ).astype(NPBF16)

    col = lambda b, n: np.ascontiguousarray(
        np.asarray(b, dtype=np.float32).reshape(n, 128).T
    )
    bpack_l = np.zeros((128, 2 * MO + 1), dtype=np.float32)
    bpack_l[:, 0:MO] = col(
        f32("b_post_emb")[:H] @ wcat0[:H]
        + f32("b_post_emb")[H:] @ wcat0[H:]
        + f32("b_post_cat"),
        MO,
    )
    bpack_l[:, MO : 2 * MO] = col(inputs["b_spt2"], MO)
    bpack_l[:NRC, 2 * MO] = f32("b_ctx")

    wspt1_l = np.zeros((33, H), dtype=np.float32)
    wspt1_l[:32] = f32("W_spt1")
    wspt1_l[32] = f32("b_spt1")

    wins, wofs, _ = _windows(sl, sr)
    return {
        "_perm": perm,
        "_sl": sl,
        "_sr": sr,
        "_wins": wins,
        "_wofs": wofs,
        "_ectx_bf16": f32("edge_ctx").astype(NPBF16),
        "_freq_f32": f32("freq_table"),
        "_bvg": (f32("b_vis") + f32("b_gate"))[None, :],
        "wcs": wcs_l,
        "wfh": wfh_l,
        "wspt1": wspt1_l.astype(NPBF16),
        "wcg": wcg_l,
        "wvisp": wvis_l,
        "bpack": bpack_l,
    }


def kernel(**inputs) -> np.ndarray:
    global last_exec_time_ns
    trace = bool(os.environ.get("BASS_KERNEL_TRACE"))
    if trace:
        _register_ntff_hook()
    common = _prep_common(inputs)
    nc = _build(common["_sl"], common["_sr"])
    in_maps = [_prep_core(inputs, c, common) for c in range(NCORES)]
    res = run_bass_kernel_spmd(nc, in_maps, list(range(NCORES)), trace=trace)
    if trace:
        last_exec_time_ns = res.exec_time_ns
    out_sorted = np.concatenate(
        [np.asarray(res.results[c]["out_t"]).T for c in range(NCORES)], axis=0
    ).astype(np.float32)
    out = np.empty_like(out_sorted)
    out[common["_perm"]] = out_sorted
    return np.ascontiguousarray(out)
